# revision 4
# baseline (speedup 1.0000x reference)
"""DCGRU cell Trainium2 kernel (8-core data-parallel over batch).

Math (per core, B_loc=4):
  gconv(x, W, b) = sum_m (A_m x) @ W_m + b,  A = [I, S0, 2S0^2-I, S1, 2S1^2-I]
  value = sigmoid(gconv1(concat(inp, hx)));  r, u = split(value)
  c = tanh(gconv2(concat(inp, r*hx)));  new = u*hx + (1-u)*c

Device layout:
  Diffusion runs as out[n_tile,(b,c)] += ST[m-chunk, n-tile]^T @ x[m-chunk,(b,c)]
  with fp16 operands, fp32 PSUM.  x1h := 2*S*x0 is stored scaled (W rows for
  m in {1,3} halved on host) so the Chebyshev combine is one vector op.
  Dense stage consumes channel-major xT tiles built by PE transposes; the bias
  is a ones-row appended to x0T.  gconv2 reuses the input-channel diffusion
  from gconv1 (those channels don't change) and overwrites only u-columns.
"""

import sys

if "/opt/trn_rl_repo" not in sys.path:
    sys.path.insert(0, "/opt/trn_rl_repo")

import numpy as np

import concourse.bass as bass
import concourse.mybir as mybir
import concourse.tile as tile
from concourse.bass_utils import run_bass_kernel_spmd
from concourse.masks import make_identity

F16 = mybir.dt.float16
F32 = mybir.dt.float32

N = 2048          # nodes
U = 64            # units
D = 2             # input dim
C = D + U         # 66 channels after concat
M = 5             # diffusion matrices
B = 32            # global batch
NCORES = 8
BL = B // NCORES  # 4 per-core batch
NT = N // 128     # 16 node tiles
NQ = 4            # dense-stage quarters
QT = NT // NQ     # tiles per quarter


def _split_drain_waits(nc):
    """This walrus build accepts only one sync-wait per instruction on several
    ISA formats; hoist extra waits onto single-wait NoOps placed before (same
    engine, so program order preserves the semantics)."""
    cnt = 0
    for f in nc.m.functions:
        for blk in f.blocks:
            new = []
            for inst in blk.instructions:
                si = inst.sync_info
                if si is not None and len(si.on_wait) > 1:
                    waits = list(si.on_wait)
                    for w in waits[:-1]:
                        cnt += 1
                        n = mybir.InstNoOp(name=f"I-dsplit-{cnt}", ins=[], outs=[])
                        n.engine = inst.engine
                        n.sync_info = mybir.SyncInfo(on_wait=[w], on_update=[])
                        new.append(n)
                    inst.sync_info = mybir.SyncInfo(
                        on_wait=[waits[-1]], on_update=list(si.on_update)
                    )
                new.append(inst)
            blk.instructions = new
    return cnt


def _build_nc():
    nc = bass.Bass()

    # DRAM parameters (host-prepped layouts)
    d_st0 = nc.dram_tensor("st0", [NT, 128, N], F16, kind="ExternalInput")
    d_st1 = nc.dram_tensor("st1", [NT, 128, NT, 128], F16, kind="ExternalInput")
    d_x0 = nc.dram_tensor("x0h", [NT, 128, BL, C], F16, kind="ExternalInput")
    d_x0t = nc.dram_tensor("x0t", [C + 1, BL, N], F16, kind="ExternalInput")
    d_hxf = nc.dram_tensor("hxf", [NT, 128, BL, U], F32, kind="ExternalInput")
    d_wru = nc.dram_tensor("wru", [C + 1, M, 2 * U], F16, kind="ExternalInput")
    d_wc = nc.dram_tensor("wc", [C + 1, M, U], F16, kind="ExternalInput")
    d_out = nc.dram_tensor("out", [NT, 128, BL, U], F32, kind="ExternalOutput")

    with tile.TileContext(nc) as tc:
        with (
            tc.tile_pool(name="const", bufs=1) as const_pool,
            tc.tile_pool(name="xbufs", bufs=1) as xbufs,
            tc.tile_pool(name="xtq", bufs=1) as xtq_pool,
            tc.tile_pool(name="st1s", bufs=2) as st1_pool,
            tc.tile_pool(name="outp", bufs=2) as out_pool,
            tc.tile_pool(name="cbuf", bufs=2) as c_pool,
            tc.tile_pool(name="dps", bufs=3, space="PSUM") as diff_ps,
            tc.tile_pool(name="tps", bufs=2, space="PSUM") as tr_ps,
            tc.tile_pool(name="nps", bufs=2, space="PSUM") as dense_ps,
        ):
            # ---- resident constants ----
            st0 = const_pool.tile([128, NT, N], F16)
            nc.sync.dma_start(out=st0, in_=d_st0[:].rearrange("t p n -> p t n"))
            ident = const_pool.tile([128, 128], F16)
            make_identity(nc, ident)
            wru = const_pool.tile([C + 1, M, 2 * U], F16)
            nc.sync.dma_start(out=wru, in_=d_wru[:, :, :])
            wc = const_pool.tile([C + 1, M, U], F16)
            nc.sync.dma_start(out=wc, in_=d_wc[:, :, :])

            x0 = xbufs.tile([128, NT, BL, C], F16, tag="x0")
            nc.sync.dma_start(out=x0, in_=d_x0[:].rearrange("t p b c -> p t (b c)"))
            hxf = xbufs.tile([128, NT, BL, U], F32, tag="hxf")
            nc.sync.dma_start(out=hxf, in_=d_hxf[:].rearrange("t p b u -> p t (b u)"))

            # diffusion outputs (m=1..4), full 66 channels, fp16
            xh = [xbufs.tile([128, NT, BL, C], F16, tag=f"xh{i}", name=f"xh{i}") for i in range(4)]
            # gconv2 state r*hx (u-columns only)
            xhp = xbufs.tile([128, NT, BL, U], F16, tag="xhp")
            r_sb = xbufs.tile([128, NT, BL, U], F16, tag="r")
            u_sb = xbufs.tile([128, NT, BL, U], F32, tag="u")
            # xT quarter buffers (channel-major, 67 rows: 66 ch + ones)
            xt = [xtq_pool.tile([C + 1, BL, QT * 128], F16, tag=f"xt{m}", name=f"xt{m}") for m in range(M)]

            def diffusion(gi):
                """4 S-applications; writes xh[0..3] (u-cols only when gi=1)."""
                nfree = C if gi == 0 else U
                x_first = x0 if gi == 0 else xhp
                for s in range(2):
                    for hop in range(2):
                        dst = xh[2 * s + hop]
                        for nt in range(NT):
                            if s == 1:
                                slab = st1_pool.tile([128, NT, 128], F16, tag="slab")
                                nc.sync.dma_start(out=slab, in_=d_st1[nt])
                            ps = diff_ps.tile([128, BL, nfree], F32, tag="dps")
                            for mc in range(NT):
                                if s == 0:
                                    lhsT = st0[:, mc, nt * 128:(nt + 1) * 128]
                                else:
                                    lhsT = slab[:, mc, :]
                                if hop == 0:
                                    if gi == 0:
                                        rhs = x_first[:, mc, :, :]
                                    else:
                                        rhs = x_first[:, mc, :, :]
                                else:
                                    prev = xh[2 * s]
                                    if gi == 0:
                                        rhs = prev[:, mc, :, :]
                                    else:
                                        rhs = prev[:, mc, :, 0:U]
                                nc.tensor.matmul(
                                    ps, lhsT, rhs, start=(mc == 0), stop=(mc == NT - 1)
                                )
                            if gi == 0:
                                dst_ap = dst[:, nt, :, :]
                                sub = x0[:, nt, :, :]
                            else:
                                dst_ap = dst[:, nt, :, 0:U]
                                sub = xhp[:, nt, :, :]
                            if hop == 0:
                                # x1h = 2 * (S x)
                                nc.vector.tensor_scalar_mul(dst_ap, ps, 2.0)
                            else:
                                # x2 = (S x1h) - x0
                                nc.vector.scalar_tensor_tensor(
                                    out=dst_ap,
                                    in0=ps,
                                    scalar=1.0,
                                    in1=sub,
                                    op0=mybir.AluOpType.mult,
                                    op1=mybir.AluOpType.subtract,
                                )

            def dense_quarters(gi):
                w_sb = wru if gi == 0 else wc
                osz = 2 * U if gi == 0 else U
                for q in range(NQ):
                    qs = slice(q * QT * 128, (q + 1) * QT * 128)
                    # xt[0]: rows 0:2 and 66 from DRAM; u-rows from host (g1) or
                    # xhp transposes (g2)
                    if gi == 0:
                        nc.sync.dma_start(out=xt[0][:, :, :], in_=d_x0t[:, :, qs])
                    else:
                        nc.sync.dma_start(out=xt[0][U:C + 1, :, :], in_=d_x0t[U:C + 1, :, qs])
                        for ntl in range(QT):
                            nt = q * QT + ntl
                            pst = tr_ps.tile([U, BL, 128], F16, tag="tps")
                            for b in range(BL):
                                nc.tensor.transpose(
                                    pst[:, b, :], xhp[:, nt, b, :], ident
                                )
                            nc.vector.tensor_copy(
                                xt[0][0:U, :, ntl * 128:(ntl + 1) * 128], pst
                            )
                    # xt[1..4] via PE transposes of diffusion outputs
                    for m in range(1, M):
                        src = xh[m - 1]
                        for ntl in range(QT):
                            nt = q * QT + ntl
                            pst = tr_ps.tile([C, BL, 128], F16, tag="tps")
                            for b in range(BL):
                                nc.tensor.transpose(
                                    pst[:, b, :], src[:, nt, b, :], ident
                                )
                            nc.vector.tensor_copy(
                                xt[m][0:C, :, ntl * 128:(ntl + 1) * 128], pst
                            )
                    # dense matmuls + activation + epilogue
                    for ntl in range(QT):
                        nt = q * QT + ntl
                        nsl = slice(ntl * 128, (ntl + 1) * 128)
                        dps = dense_ps.tile([128, BL, osz], F32, tag="nps")
                        for b in range(BL):
                            for m in range(M):
                                k = C + 1 if m == 0 else C
                                nc.tensor.matmul(
                                    dps[:, b, :],
                                    xt[m][0:k, b, nsl],
                                    w_sb[0:k, m, :],
                                    start=(m == 0),
                                    stop=(m == M - 1),
                                )
                        if gi == 0:
                            # r (cols 0:U) -> fp16, u (cols U:2U) -> fp32
                            nc.scalar.activation(
                                out=r_sb[:, nt, :, :],
                                in_=dps[:, :, 0:U],
                                func=mybir.ActivationFunctionType.Sigmoid,
                            )
                            nc.scalar.activation(
                                out=u_sb[:, nt, :, :],
                                in_=dps[:, :, U:2 * U],
                                func=mybir.ActivationFunctionType.Sigmoid,
                            )
                            nc.vector.tensor_mul(
                                xhp[:, nt, :, :], r_sb[:, nt, :, :], x0[:, nt, :, 0:U]
                            )
                        else:
                            cb = c_pool.tile([128, BL, U], F32, tag="cb")
                            nc.scalar.activation(
                                out=cb,
                                in_=dps,
                                func=mybir.ActivationFunctionType.Tanh,
                            )
                            # new = c + u*(hx - c)
                            tmp = c_pool.tile([128, BL, U], F32, tag="tmp")
                            nc.vector.tensor_sub(tmp, hxf[:, nt, :, :], cb)
                            nc.vector.tensor_mul(tmp, u_sb[:, nt, :, :], tmp)
                            ob = out_pool.tile([128, BL, U], F32, tag="ob")
                            nc.vector.tensor_add(ob, tmp, cb)
                            nc.sync.dma_start(out=d_out[nt], in_=ob)

            diffusion(0)
            dense_quarters(0)
            diffusion(1)
            dense_quarters(1)

    _split_drain_waits(nc)
    return nc


_NC_CACHE = None


def _get_nc():
    global _NC_CACHE
    if _NC_CACHE is None:
        _NC_CACHE = _build_nc()
    return _NC_CACHE


def _prep_host(inputs, hx, support0, support1, W_ru, b_ru, W_c, b_c):
    f16 = np.float16
    inp = inputs.reshape(B, N, D).astype(np.float32)
    hx3 = hx.reshape(B, N, U).astype(np.float32)
    x0_full = np.concatenate([hx3, inp], axis=2)  # [B, N, C] fp32, u-first

    st0 = np.ascontiguousarray(support0.T).astype(f16).reshape(NT, 128, N)
    st1 = np.ascontiguousarray(
        support1.T.reshape(NT, 128, NT, 128).transpose(2, 1, 0, 3)
    ).astype(f16)

    def prep_w(W, bvec, osz):
        w = W.reshape(C, M, osz).astype(np.float32)
        w = np.concatenate([w[D:], w[:D]], axis=0).copy()  # u-first rows
        w[:, 1, :] *= 0.5
        w[:, 3, :] *= 0.5
        wf = np.zeros((C + 1, M, osz), np.float32)
        wf[:C] = w
        wf[C, 0, :] = bvec
        return wf.astype(f16)

    wru = prep_w(W_ru, b_ru, 2 * U)
    wcc = prep_w(W_c, b_c, U)

    in_maps = []
    for c in range(NCORES):
        cs = slice(c * BL, (c + 1) * BL)
        x0c = x0_full[cs]                                   # [BL, N, C]
        x0h = np.ascontiguousarray(
            x0c.transpose(1, 0, 2).reshape(NT, 128, BL, C)
        ).astype(f16)
        x0t = np.concatenate(
            [x0c.transpose(2, 0, 1), np.ones((1, BL, N), np.float32)], axis=0
        ).astype(f16)                                        # [C+1, BL, N]
        hxf = np.ascontiguousarray(
            hx3[cs].transpose(1, 0, 2).reshape(NT, 128, BL, U)
        ).astype(np.float32)
        in_maps.append(
            {
                "st0": st0,
                "st1": st1,
                "x0h": x0h,
                "x0t": np.ascontiguousarray(x0t),
                "hxf": hxf,
                "wru": wru,
                "wc": wcc,
            }
        )
    return in_maps


def kernel(inputs, hx, support0, support1, W_ru, b_ru, W_c, b_c, _trace=False,
           _tmpdir=None):
    nc = _get_nc()
    in_maps = _prep_host(
        inputs, hx, support0, support1, W_ru, b_ru, W_c, b_c
    )
    res = run_bass_kernel_spmd(
        nc, in_maps, core_ids=list(range(NCORES)), trace=_trace, tmpdir=_tmpdir
    )
    out = np.empty((B, N * U), np.float32)
    for c in range(NCORES):
        od = res.results[c]["out"]  # [NT, 128, BL, U]
        out[c * BL:(c + 1) * BL] = (
            od.transpose(2, 0, 1, 3).reshape(BL, N * U)
        )
    kernel._last_result = res
    return out
